# revision 62
# baseline (speedup 1.0000x reference)
"""Causal self-attention (B=4, T=2048, C=1024, H=16) on 8 TRN2 NeuronCores.

Sharding: core = (batch, head-group) — data parallel over the 4 batches,
tensor parallel over 2 groups of 8 heads (Megatron-style column/row split of
the qkv / out projections).  Each core computes a [T, C] partial of the out
projection for its head group; the host sums the two partials per batch and
adds b_out, so no device collectives are needed.

All matmul operands are bf16 (PE streams 1 col/cycle regardless of dtype,
so bf16 costs nothing on the PE but halves SBUF/DMA traffic, enables FWL
weight loads, and lets an S^T block-pair share one PSUM bank).  PSUM
accumulators stay fp32 except the S^T logits, which are written bf16 so the
even/odd head pair packs into a single bank and one ScalarE Exp covers both
(halving ACT instruction overhead, the phase-B bottleneck).

Device kernel, per tq-slab of 512:
  A(s) q^T,k^T = (W_qk chunk)^T @ x^T slab; v = x @ W_v (+bias via K=1 ones
       matmul).  PSUM evictions on DVE (tensor_scalar: scale+bias).
  B(h,s) S^T pair = k^T-block @ q^T-slab (K=64; odd heads in partitions
       64-127 so even/odd pairs overlap in distinct PE row groups), one
       merged Exp per pair on ScalarE, causal 0/1 mask post-exp on GpSimd
       for the diagonal subtiles, then P@V with lhsT=[v | 1] so the softmax
       denominator accumulates for free as PSUM row D.  Blocks run in
       DESCENDING tk order so the masked diagonal work is off the slab tail
       (has_written handles the ragged first write).  Normalization without
       any DRAM bounce: d -> SBUF (DVE), K=1 ones-matmul broadcasts d into
       partitions 64-127 of the same PSUM bank, reciprocal_approx_fast +
       tensor_mul write the normalized y^T straight into yT_sb.
  C(s) out partial = y^T chunks @ W_out chunks.

To keep the PE instruction queue free of multi-us stalls (which re-trip the
HAM clock throttle and halve the PE clock), A(s+1) and C(s-1) are emitted as
work units INTERLEAVED into B(s)'s block loop, so the statically scheduled
PE stream always has projection matmuls to chew on while ScalarE works
through the Exp backlog.
"""

import os
import sys
from contextlib import ExitStack

import numpy as np
import ml_dtypes

for _p in ("/opt/trn_rl_repo", "/root/.axon_site/_ro/trn_rl_repo"):
    if os.path.isdir(_p) and _p not in sys.path:
        sys.path.append(_p)

import concourse.bacc as bacc
import concourse.bass as bass
import concourse.tile as tile
from concourse import mybir
from concourse.bass_utils import run_bass_kernel_spmd
from concourse.masks import make_upper_triangular

AF = mybir.ActivationFunctionType
ALU = mybir.AluOpType
F32 = mybir.dt.float32
F32R = mybir.dt.float32r
BF16 = mybir.dt.bfloat16

P = 128
SLAB = 512

B, T, C, H, D = 4, 2048, 1024, 16, 64
N_CORES = 8
N_GROUPS = 2          # head groups (tensor-parallel degree per batch)
HL = H // N_GROUPS    # heads per core
CL = HL * D           # local qkv width


def _build_nc(loop_reps=None, debug_dump=False):
    NCK = C // P
    MQK = 2 * CL // P
    MQ = MQK // 2
    TT = T // P
    NS = T // SLAB
    YC = CL // P
    W_OUT = min(SLAB, C)
    NOUT = C // W_OUT
    scale = 1.0 / np.sqrt(D)

    nc = bacc.Bacc("TRN2", target_bir_lowering=False, debug=False,
                   num_devices=N_CORES)
    xT = nc.dram_tensor("xT", [C, T], BF16, kind="ExternalInput")
    wqk = nc.dram_tensor("wqk", [C, 2 * CL], BF16, kind="ExternalInput")
    wv = nc.dram_tensor("wv", [C, CL], BF16, kind="ExternalInput")
    wout = nc.dram_tensor("wout", [CL, C], BF16, kind="ExternalInput")
    bqk = nc.dram_tensor("bqk", [P, MQK], F32, kind="ExternalInput")
    bv = nc.dram_tensor("bv", [1, CL], BF16, kind="ExternalInput")
    outp = nc.dram_tensor("outp", [T, C], BF16, kind="ExternalOutput")
    scr = nc.dram_tensor("scr", [H * NS, SLAB], BF16)
    if debug_dump:
        dbg_d = nc.dram_tensor("dbg_d", [8, SLAB], F32, kind="ExternalOutput")
        dbg_b = nc.dram_tensor("dbg_b", [8, SLAB], F32, kind="ExternalOutput")
        dbg_y = nc.dram_tensor("dbg_y", [P, 4 * SLAB], F32,
                               kind="ExternalOutput")

    with tile.TileContext(nc) as tc, ExitStack() as ctx:
        pool = lambda name, bufs, **kw: ctx.enter_context(
            tc.tile_pool(name=name, bufs=bufs, **kw))

        const = pool("const", 1)
        kp = pool("kp", 1)
        vp = pool("vp", 1)
        wqkp = pool("wqkp", 1)
        wvp = pool("wvp", 1)
        woutp = pool("woutp", 1)
        xtp = pool("xt", 3)
        qp = pool("qp", 2)
        yTp = pool("yTp", 2)
        expp = pool("expp", 8)
        dp = pool("dp", 2)
        binvp = pool("binvp", 2)
        otp = pool("ot", 6)
        psPO = pool("psPO", 2, space="PSUM")
        psS = pool("psS", 2, space="PSUM")
        psY = pool("psY", 1, space="PSUM")

        k_sb = kp.tile([P, MQ, T], BF16)
        v_sb = vp.tile([P, TT, HL, D + 1], BF16)
        wqk_sb = wqkp.tile([P, NCK, 2 * CL], BF16)
        wv_sb = wvp.tile([P, NCK, CL], BF16)
        wout_sb = woutp.tile([P, YC, C], BF16)
        bqk_sb = const.tile([P, MQK], F32)
        bv_sb = const.tile([1, CL], BF16)
        mask_f = const.tile([P, P], F32)
        mask01 = const.tile([P, P], BF16)
        onescr = const.tile([P, TT * HL], F32)
        ones64b = const.tile([1, 64], BF16)

        warm_f = const.tile([P, SLAB], F32)
        cwarm = const.tile([P, SLAB], BF16)
        wscr = const.tile([1, 1], F32)

        def emit_init_dmas():
            # The startup is DMA-bandwidth-bound: the first S pair needs
            # only the q0/k0 column slices of wqk (0.5 MB), so ship those
            # first, then the rest of wqk, then v / out-projection weights.
            for c in range(NCK):
                nc.sync.dma_start(out=wqk_sb[:, c, 0:P],
                                  in_=wqk[c * P:(c + 1) * P, 0:P])
                nc.sync.dma_start(
                    out=wqk_sb[:, c, MQ * P:(MQ + 1) * P],
                    in_=wqk[c * P:(c + 1) * P, MQ * P:(MQ + 1) * P])
            nc.sync.dma_start(out=bqk_sb[:, :], in_=bqk[:, :])
            nc.sync.dma_start(out=bv_sb[:, :], in_=bv[:, :])
            for c in range(NCK):
                nc.sync.dma_start(out=wqk_sb[:, c, P:MQ * P],
                                  in_=wqk[c * P:(c + 1) * P, P:MQ * P])
                nc.sync.dma_start(
                    out=wqk_sb[:, c, (MQ + 1) * P:],
                    in_=wqk[c * P:(c + 1) * P, (MQ + 1) * P:])
            for c in range(NCK):
                nc.sync.dma_start(out=wv_sb[:, c, :],
                                  in_=wv[c * P:(c + 1) * P, :])
            for c in range(YC):
                nc.sync.dma_start(out=wout_sb[:, c, :],
                                  in_=wout[c * P:(c + 1) * P, :])

        # mask01[p, f] = 1 if f >= p else 0  (S^T visibility: tq >= tk).
        make_upper_triangular(nc, mask_f[:, :], val=1.0, diag=True)
        nc.vector.tensor_copy(mask01[:, :], mask_f[:, :])
        nc.vector.memset(onescr[:, :], 1.0)
        nc.vector.tensor_copy(
            v_sb[:, :, :, D],
            onescr[:, :].rearrange("p (t h) -> p t h", h=HL))
        nc.vector.tensor_copy(ones64b[0:1, :], onescr[0:1, 0:64])
        nc.vector.memset(warm_f[:, :], 1.0)
        nc.vector.tensor_copy(cwarm[:, :], warm_f[:, :])
        ones1 = v_sb[0:1, :, :, D].rearrange("u t h -> u (t h)")

        def emit_warmup(n_mm):
            # Keep the PE busy on const data while the weight/x DMAs land,
            # so the HAM clock gate is at 8/8 when real matmuls start.
            # Full-array K=128 matmuls — HAM watches PE activity, so thin
            # matmuls don't register as busy.
            ps_w = psS.tile([P, 2, SLAB], F32, tag="s")
            for _ in range(n_mm):
                nc.tensor.matmul(ps_w[:, 0, :], mask01[:, :],
                                 cwarm[:, :], start=True, stop=True)
            nc.vector.tensor_copy(wscr[0:1, 0:1], ps_w[0:1, 0, 0:1])

        def emit_xt_load(s):
            t0 = s * SLAB
            xt = xtp.tile([P, NCK, SLAB], BF16)
            for c in range(NCK):
                nc.sync.dma_start(out=xt[:, c, :],
                                  in_=xT[c * P:(c + 1) * P, t0:t0 + SLAB])
            return xt

        def make_a_units(s, xt):
            """Projection work units for slab s, split to <=~850ns of PE
            work each so an interleaved unit never starves ScalarE."""
            t0 = s * SLAB
            q_sb = qp.tile([P, MQ, SLAB], BF16)
            HC = NCK // 2

            def qk_units(m):
                cell = {}

                def run_a():
                    ps = psPO.tile([P, SLAB], F32, tag="ps")
                    cell["ps"] = ps
                    for c in range(HC):
                        nc.tensor.matmul(
                            ps[:, :],
                            wqk_sb[:, c, m * P:(m + 1) * P],
                            xt[:, c, :],
                            start=(c == 0), stop=False)

                def run_b():
                    ps = cell["ps"]
                    for c in range(HC, NCK):
                        nc.tensor.matmul(
                            ps[:, :],
                            wqk_sb[:, c, m * P:(m + 1) * P],
                            xt[:, c, :],
                            start=False, stop=(c == NCK - 1))
                    dst = (q_sb[:, m, :] if m < MQ
                           else k_sb[:, m - MQ, t0:t0 + SLAB])
                    sc = scale if m < MQ else 1.0
                    nc.vector.tensor_scalar(
                        dst, ps[:, :], sc, bqk_sb[:, m:m + 1],
                        op0=ALU.mult, op1=ALU.add)

                return [run_a, run_b]

            def v_units(sub):
                cell = {}

                def run_a():
                    ps = psPO.tile([P, CL], F32, tag="ps")
                    cell["ps"] = ps
                    for c in range(HC):
                        nc.tensor.matmul(
                            ps[:, :],
                            xt[:, c, sub * P:(sub + 1) * P],
                            wv_sb[:, c, :],
                            start=(c == 0), stop=False)

                def run_b():
                    tt = s * (SLAB // P) + sub
                    ps = cell["ps"]
                    for c in range(HC, NCK):
                        nc.tensor.matmul(
                            ps[:, :],
                            xt[:, c, sub * P:(sub + 1) * P],
                            wv_sb[:, c, :],
                            start=False, stop=False)
                    nc.tensor.matmul(
                        ps[:, :], ones1[:, :],
                        bv_sb[0:1, :], start=False, stop=True)
                    nc.vector.tensor_copy(
                        v_sb[:, tt, :, 0:D],
                        ps[:, :].rearrange("p (h d) -> p h d", d=D))

                return [run_a, run_b]

            # Dependency-friendly order: (q_m, k_m) pairs so B of this slab
            # can start per-head-pair as soon as its q/k/v tiles land.
            units = qk_units(0) + qk_units(MQ)
            for sub in range(SLAB // P):
                units += v_units(sub)
            for m in range(1, MQ):
                units += qk_units(m) + qk_units(MQ + m)
            return q_sb, units

        def make_c_units(s, yT_sb, final=False):
            """Out-projection work units for slab s (inputs: yT tile of s).
            final=True: the attention pipeline is drained, so borrow the
            idle psS/psY banks as extra accumulators and spread the store
            DMAs over both HWDGE queues."""
            t0 = s * SLAB
            alloc_n = [0]

            def alloc_ps():
                if not final:
                    return psPO.tile([P, W_OUT], F32, tag="ps", name="cps")
                k = alloc_n[0] % 4
                alloc_n[0] += 1
                if k == 0:
                    return psPO.tile([P, W_OUT], F32, tag="ps", name="cps")
                if k == 1:
                    return psS.tile([P, 2, SLAB], F32, tag="s",
                                    name="cps")[:, 0, :]
                if k == 2:
                    return psY.tile([P, SLAB], F32, tag="py0", name="cps")
                return psY.tile([P, SLAB], F32, tag="py1", name="cps")

            def c_units(sub, n, on_scalar):
                cell = {}
                n0 = n * W_OUT

                def run_a():
                    ps = alloc_ps()
                    cell["ps"] = ps
                    for c in range(YC // 2):
                        nc.tensor.matmul(
                            ps[:, :],
                            yT_sb[:, c, sub * P:(sub + 1) * P],
                            wout_sb[:, c, n0:n0 + W_OUT],
                            start=(c == 0), stop=False)

                def run_b():
                    ps = cell["ps"]
                    for c in range(YC // 2, YC):
                        nc.tensor.matmul(
                            ps[:, :],
                            yT_sb[:, c, sub * P:(sub + 1) * P],
                            wout_sb[:, c, n0:n0 + W_OUT],
                            start=False, stop=(c == YC - 1))
                    ot = otp.tile([P, W_OUT], BF16)
                    # alternate evictions between ScalarE (Copy: no table)
                    # and DVE so neither engine serializes the C stream
                    if on_scalar:
                        nc.scalar.copy(ot[:, :], ps[:, :])
                    else:
                        nc.vector.tensor_copy(ot[:, :], ps[:, :])
                    eng = nc.scalar if (final and on_scalar) else nc.sync
                    eng.dma_start(
                        out=outp[t0 + sub * P:t0 + (sub + 1) * P,
                                 n0:n0 + W_OUT],
                        in_=ot[:, :])

                return [run_a, run_b]

            units = []
            for sub in range(SLAB // P):
                for n in range(NOUT):
                    units += c_units(sub, n, (sub * NOUT + n) % 2 == 0)
            return units

        def emit_b(s, q_sb, units, inject=None):
            """Attention for slab s; drains `units` into the block loop.

            The S^T matmul pair for block b+1 is emitted BEFORE the P@V
            pair of block b, so the next Exp's input is at the head of
            the PE queue and ScalarE stays saturated."""
            t0 = s * SLAB
            nblk = (s + 1) * SLAB // P
            nsteps = 4 * nblk
            yT_sb = yTp.tile([P, YC, SLAB], BF16)
            ucur = 0
            ucredit = 0.0
            upd = len(units) / nsteps

            # Slab 0 runs its blocks ASCENDING (every slab-0 block starts
            # at its own visibility edge, so the has_written ragged-start
            # logic is order-independent): head-pair 0 can then start
            # after just q0/k0, with each v tile injected right before
            # the P@V that first needs it.
            if s == 0:
                blocks = list(range(nblk))
            else:
                blocks = list(range(nblk - 1, -1, -1))
            inject = inject or {}
            steps = [(hp, bi) for hp in range(HL // 2) for bi in range(nblk)]
            ps_tiles = {}

            def emit_s(k):
                # S^T pair for step k (idempotent).  Hoisted ahead of the
                # P@V / eviction-chain matmuls in the PE stream so the Exp
                # pipeline on ScalarE never stalls behind them; psS bufs=2
                # turns an early emission into a plain WAR dependency.
                if k >= len(steps) or k in ps_tiles:
                    return
                hp, bi = steps[k]
                tk0 = blocks[bi] * P
                vis = max(0, tk0 - t0)
                ps = psS.tile([P, 2, SLAB], F32, tag="s")
                for i in range(2):
                    row0 = i * 64
                    nc.tensor.matmul(
                        ps[:, i, vis:SLAB],
                        k_sb[row0:row0 + 64, hp, tk0:tk0 + P],
                        q_sb[row0:row0 + 64, hp, vis:SLAB],
                        start=True, stop=True,
                        tile_position=(row0, 0))
                ps_tiles[k] = ps

            emit_s(0)
            pys = None
            for k, (hp, bi) in enumerate(steps):
                b = blocks[bi]
                if bi == 0:
                    py0 = psY.tile([P, SLAB], F32, tag="py0")
                    py1 = psY.tile([P, SLAB], F32, tag="py1")
                    pys = (py0, py1)
                tk0 = b * P
                off = tk0 - t0
                vis = max(0, off)
                ep = expp.tile([P, 2, SLAB], BF16)
                nc.scalar.activation(ep[:, :, vis:SLAB],
                                     ps_tiles.pop(k)[:, :, vis:SLAB], AF.Exp)
                emit_s(k + 1)
                if off >= 0:
                    for i in range(2):
                        nc.gpsimd.tensor_mul(
                            ep[:, i, off:off + P], ep[:, i, off:off + P],
                            mask01[:, :])
                if bi == nblk - 1:
                    # head-pair boundary: hoist the NEXT pair's second S
                    # too, so it precedes the eviction chain's broadcast
                    # matmuls in the PE queue
                    emit_s(k + 2)
                for u in inject.get((hp, bi), ()):
                    u()
                for i in range(2):
                    nc.tensor.matmul(
                        pys[i][0:D + 1, vis:SLAB],
                        v_sb[:, b, 2 * hp + i, 0:D + 1],
                        ep[:, i, vis:SLAB],
                        start=(bi == 0), stop=(bi == nblk - 1))
                ucredit += upd
                while ucur < len(units) and ucur < int(ucredit):
                    units[ucur]()
                    ucur += 1
                if bi != nblk - 1:
                    continue
                # Normalize + evict: reciprocal of the denominator row,
                # K=1 ones-matmul broadcasts it into partitions 64-127 of
                # the same py bank, then the raw numerator is copied out
                # and multiplied in place.
                for i in range(2):
                    row0 = i * 64
                    d_sb = dp.tile([1, SLAB], F32, tag=f"d{i}")
                    nc.vector.tensor_copy(d_sb[0:1, :], pys[i][D:D + 1, :])
                    rin = dp.tile([1, SLAB], F32, tag=f"r{i}")
                    nc.vector.reciprocal_approx_fast(rin[0:1, :],
                                                     d_sb[0:1, :])
                    rb = dp.tile([1, SLAB], BF16, tag=f"rb{i}")
                    nc.vector.tensor_copy(rb[0:1, :], rin[0:1, :])
                    nc.tensor.matmul(
                        pys[i][64:128, :], ones64b[0:1, :], rb[0:1, :],
                        start=True, stop=True, tile_position=(0, 64))
                    nc.vector.tensor_copy(
                        yT_sb[row0:row0 + 64, hp, :], pys[i][0:D, :])
                    nc.vector.tensor_mul(
                        yT_sb[row0:row0 + 64, hp, :],
                        yT_sb[row0:row0 + 64, hp, :], pys[i][64:128, :])
                    if debug_dump and s == 0:
                        di = 2 * hp + i
                        nc.sync.dma_start(out=dbg_d[di:di + 1, :],
                                          in_=d_sb[0:1, :])
                        nc.sync.dma_start(out=dbg_b[di:di + 1, :],
                                          in_=rin[0:1, :])
            while ucur < len(units):
                units[ucur]()
                ucur += 1
            if debug_dump and s == 0:
                ysc = binvp.tile([P, 4 * SLAB], F32, tag="ydbg")
                nc.vector.tensor_copy(
                    ysc[:, :].rearrange("p (c t) -> p c t", c=4),
                    yT_sb[:, :, :])
                nc.sync.dma_start(out=dbg_y[:, :], in_=ysc[:, :])
            return yT_sb

        def weave(l1, l2):
            out = []
            i = j = 0
            n1, n2 = len(l1), len(l2)
            while i < n1 or j < n2:
                if j >= n2 or (i < n1 and i * (n2 + 1) <= j * (n1 + 1)):
                    out.append(l1[i])
                    i += 1
                else:
                    out.append(l2[j])
                    j += 1
            return out

        def body():
            NSL = T // SLAB
            xts = {0: emit_xt_load(0)}
            emit_init_dmas()
            emit_warmup(32)
            xts[1] = emit_xt_load(1)
            q_sb, a_units = make_a_units(0, xts[0])
            # run just q0/k0 of A(0) inline; the v tiles are injected
            # just-in-time into B(0)'s ascending block loop, and the rest
            # of A(0) joins the interleave list
            for u in a_units[:4]:
                u()
            inj0 = {(0, bi): a_units[4 + 2 * bi: 6 + 2 * bi]
                    for bi in range(4)}
            carry = a_units[12:]
            yts_done = {}
            for s in range(NSL):
                # prefetch x two slabs ahead so these loads sit in the DMA
                # queue BEFORE the out-projection stores (which can wait on
                # their evictions and block the queue head)
                if s + 2 < NSL:
                    xts[s + 2] = emit_xt_load(s + 2)
                if s + 1 < NSL:
                    q_next, a_units = make_a_units(s + 1, xts[s + 1])
                else:
                    q_next = None
                    a_units = []
                c_units = (make_c_units(s - 1, yts_done[s - 1])
                           if s >= 1 else [])
                units = carry + weave(a_units, c_units)
                carry = []
                yts_done[s] = emit_b(s, q_sb, units,
                                     inject=(inj0 if s == 0 else None))
                q_sb = q_next
            for u in make_c_units(NSL - 1, yts_done[NSL - 1], final=True):
                u()

        if loop_reps is None:
            body()
        else:
            with tc.For_i(0, loop_reps, 1):
                body()

    nc.compile()
    return nc


_NC_CACHE = None


def _get_nc():
    global _NC_CACHE
    if _NC_CACHE is None:
        _NC_CACHE = _build_nc()
    return _NC_CACHE


def make_in_maps(x, W_qkv, b_qkv, W_out):
    scale = 1.0 / np.sqrt(D)
    MQK = 2 * CL // P
    bf = ml_dtypes.bfloat16
    in_maps = []
    for core in range(N_CORES):
        b, hg = divmod(core, N_GROUPS)
        qs = slice(hg * CL, (hg + 1) * CL)
        ks = slice(C + hg * CL, C + (hg + 1) * CL)
        vs = slice(2 * C + hg * CL, 2 * C + (hg + 1) * CL)
        bqk_cat = np.concatenate([b_qkv[qs] * scale, b_qkv[ks]])
        in_maps.append({
            "xT": np.ascontiguousarray(x[b].T.astype(bf)),
            "wqk": np.ascontiguousarray(
                np.concatenate([W_qkv[:, qs], W_qkv[:, ks]],
                               axis=1).astype(bf)),
            "wv": np.ascontiguousarray(W_qkv[:, vs].astype(bf)),
            "wout": np.ascontiguousarray(
                W_out[hg * CL:(hg + 1) * CL, :].astype(bf)),
            "bqk": np.ascontiguousarray(bqk_cat.reshape(MQK, P).T
                                        .astype(np.float32)),
            "bv": np.ascontiguousarray(b_qkv[vs].reshape(1, CL).astype(bf)),
        })
    return in_maps


def kernel(x, W_qkv, b_qkv, W_out, b_out):
    x = np.asarray(x, dtype=np.float32)
    W_qkv = np.asarray(W_qkv, dtype=np.float32)
    b_qkv = np.asarray(b_qkv, dtype=np.float32)
    W_out = np.asarray(W_out, dtype=np.float32)
    b_out = np.asarray(b_out, dtype=np.float32)

    nc = _get_nc()
    in_maps = make_in_maps(x, W_qkv, b_qkv, W_out)
    res = run_bass_kernel_spmd(nc, in_maps, core_ids=list(range(N_CORES)))

    out = np.empty((B, T, C), dtype=np.float32)
    for b in range(B):
        out[b] = (res.results[N_GROUPS * b]["outp"].astype(np.float32)
                  + res.results[N_GROUPS * b + 1]["outp"].astype(np.float32)
                  + b_out)
    return out
